# revision 13
# baseline (speedup 1.0000x reference)
"""Trainium2 Bass kernel for a separable 3D Haar DWT (nn_DWT3D).

Problem: x [2, 128, 128, 128, 4] fp32, A [128, 128] (orthonormal Haar
analysis filter bank, 2-tap stride-2). Output: subband concat
[2, 64, 64, 64, 32].

Strategy (8 NeuronCores):
- Data-parallel over (batch, channel): B*C = 8 independent [128,128,128]
  volume transforms, one per core. Host deinterleaves channels on the
  way in and assembles the subband concat on the way out (numpy).
- Per core, separable transform = three 1D passes:
    * i-axis (partition axis): dense PE matmul with lhsT = h^2 * A^T
      (the j/k butterfly scales folded into the weights).
    * j-axis: DVE butterfly reading matmul results DIRECTLY FROM PSUM
      (saves a PSUM->SBUF copy stage).
    * k-axis: DVE butterfly SBUF->SBUF.
- Streamed in 1 MB chunks; input DMA issued from the sync ring and
  output DMA from the scalar ring so the streams don't head-of-line
  block each other.
"""

import numpy as np

_N = 128
_CORES = 8
# j-counts per streamed compute group (must each be even, sum to 128).
# Smaller first/last groups shorten the pipeline ramp and tail.
_GROUPS = [4, 12] + [16] * 6 + [8, 4, 4]

_cache = {}


def _build():
    import concourse.mybir as mybir
    from concourse import bacc
    from concourse.tile import TileContext

    nc = bacc.Bacc("TRN2", target_bir_lowering=False, debug=False,
                   num_devices=_CORES)
    v = nc.dram_tensor("v", [_N, _N * _N], mybir.dt.float32,
                       kind="ExternalInput")
    w = nc.dram_tensor("w", [_N, _N], mybir.dt.float32,
                       kind="ExternalInput")
    y = nc.dram_tensor("y", [_N, _N * _N], mybir.dt.float32,
                       kind="ExternalOutput")
    f32 = mybir.dt.float32

    # Compute groups: (column offset, j-count). The last groups are smaller
    # so the pipeline tail (matmul -> butterflies -> out-DMA of the final
    # group) is short.
    groups = []
    off = 0
    for jb in _GROUPS:
        groups.append((off, jb))
        off += jb * _N
    assert off == _N * _N

    with TileContext(nc) as tc:
        with (
            tc.tile_pool(name="wpool", bufs=1) as wpool,
            tc.tile_pool(name="vin", bufs=4) as vpool,
            tc.tile_pool(name="y1e", bufs=3) as y1pool,
            tc.tile_pool(name="y2", bufs=3) as y2pool,
            tc.tile_pool(name="y3", bufs=4) as y3pool,
            tc.tile_pool(name="ps", bufs=2, space="PSUM") as pspool,
        ):
            wt = wpool.tile([_N, _N], f32)
            nc.sync.dma_start(out=wt[:], in_=w[:])
            for off, jb in groups:
                cw = jb * _N
                half = cw // 2
                vin = vpool.tile([_N, cw], f32, tag="vin")
                nc.sync.dma_start(out=vin[:], in_=v[:, off:off + cw])

                # i-axis transform on PE: ps[a, col] = sum_i w[i,a] vin[i,col]
                ps = pspool.tile([_N, cw], f32, tag="ps")
                for m in range(cw // 512):
                    nc.tensor.matmul(ps[:, m * 512:(m + 1) * 512], wt[:],
                                     vin[:, m * 512:(m + 1) * 512],
                                     start=True, stop=True)

                # ACT drains PSUM to SBUF quickly (frees the PSUM slot for
                # the next group's matmuls), then DVE butterflies in SBUF.
                y1 = y1pool.tile([_N, cw], f32, tag="y1e")
                nc.scalar.copy(out=y1[:], in_=ps[:])
                y1v = y1[:].rearrange("p (j k) -> p j k", k=_N)
                y2 = y2pool.tile([_N, cw], f32, tag="y2")
                y2L = y2[:, 0:half].rearrange("p (j k) -> p j k", k=_N)
                y2H = y2[:, half:cw].rearrange("p (j k) -> p j k", k=_N)
                nc.vector.tensor_add(out=y2L[:],
                                     in0=y1v[:, 1:jb:2, :],
                                     in1=y1v[:, 0:jb:2, :])
                nc.vector.tensor_sub(out=y2H[:],
                                     in0=y1v[:, 1:jb:2, :],
                                     in1=y1v[:, 0:jb:2, :])

                # k-axis butterfly (stride-2 pairs along k)
                y2v = y2[:].rearrange("p (t k) -> p t k", k=_N)
                y3 = y3pool.tile([_N, cw], f32, tag="y3")
                y3L = y3[:, 0:half].rearrange("p (t k) -> p t k", k=64)
                y3H = y3[:, half:cw].rearrange("p (t k) -> p t k", k=64)
                nc.vector.tensor_add(out=y3L[:],
                                     in0=y2v[:, :, 0:_N:2],
                                     in1=y2v[:, :, 1:_N:2])
                nc.vector.tensor_sub(out=y3H[:],
                                     in0=y2v[:, :, 1:_N:2],
                                     in1=y2v[:, :, 0:_N:2])

                nc.scalar.dma_start(out=y[:, off:off + cw], in_=y3[:])

    nc.compile()
    return nc


def _get_nc():
    if "nc" not in _cache:
        _cache["nc"] = _build()
    return _cache["nc"]


def _haar_structure_ok(A):
    """A must be the 2-tap stride-2 filter bank with taps (h, h) lowpass /
    (-h, h) highpass, which is what the j/k butterflies hardcode."""
    if A.shape != (_N, _N):
        return False
    h = A[0, 0]
    if not np.isfinite(h) or abs(h) < 1e-8:
        return False
    expect = np.zeros((_N, _N), dtype=np.float32)
    for i in range(_N // 2):
        expect[i, 2 * i] = h
        expect[i, 2 * i + 1] = h
        expect[_N // 2 + i, 2 * i] = -h
        expect[_N // 2 + i, 2 * i + 1] = h
    return bool(np.allclose(A, expect, rtol=1e-5, atol=1e-7))


def _reference_host(x, A):
    """Generic numpy fallback (slow) for non-Haar A."""
    y = np.einsum("ai,nijkc->najkc", A, x, optimize=True)
    y = np.einsum("bj,najkc->nabkc", A, y, optimize=True)
    y = np.einsum("dk,nabkc->nabdc", A, y, optimize=True)
    return np.moveaxis(y, -1, 1)


def _assemble(y_full, B, C):
    """Slice transformed volumes y_full [B, C, 128,128,128] into the
    reference's subband concat [B, 64, 64, 64, 8*C] (incl. the duplicated
    HHH octant the reference produces)."""
    L, H = slice(0, 64), slice(64, 128)
    bands = [(L, L, L), (H, L, L), (L, H, L), (H, H, L),
             (L, L, H), (H, H, H), (L, H, H), (H, H, H)]
    out = np.empty((B, 64, 64, 64, 8 * C), dtype=np.float32)
    for s, (sa, sb, sd) in enumerate(bands):
        out[..., s * C:(s + 1) * C] = np.moveaxis(y_full[:, :, sa, sb, sd], 1, -1)
    return out


def kernel(x, A):
    from concourse.bass_utils import run_bass_kernel_spmd

    x = np.asarray(x, dtype=np.float32)
    A = np.asarray(A, dtype=np.float32)
    B, _, _, _, C = x.shape
    assert (B, C) == (2, 4) and x.shape[1:4] == (_N, _N, _N)

    if not _haar_structure_ok(A):
        return _assemble(_reference_host(x, A), B, C)

    h = float(A[0, 0])
    # PE weights: actual A with the j/k butterfly scale h^2 folded in.
    w = np.ascontiguousarray((h * h) * A.T)

    xs = np.ascontiguousarray(np.transpose(x, (0, 4, 1, 2, 3)))
    xs = xs.reshape(_CORES, _N, _N * _N)
    in_maps = [{"v": xs[g], "w": w} for g in range(_CORES)]

    nc = _get_nc()
    res = run_bass_kernel_spmd(nc, in_maps, list(range(_CORES)))

    # Per-core output, per group [a, lhk, (lhj, tj), kk]:
    # j' = jbase/2 + tj (L) or 64 + jbase/2 + tj (H); d = 64*lhk + kk.
    y_full = np.empty((B, C, _N, _N, _N), dtype=np.float32)
    for g in range(_CORES):
        cols = res.results[g]["y"]
        vol = y_full[g // C, g % C]
        off = 0
        for jb in _GROUPS:
            z = cols[:, off:off + jb * _N].reshape(_N, 2, jb, 64)
            z = z.transpose(0, 2, 1, 3)  # [a, (lhj, tj), lhk, kk]
            mu = (off // _N) // 2
            vol[:, mu:mu + jb // 2, :] = z[:, 0:jb // 2].reshape(_N, jb // 2, _N)
            vol[:, 64 + mu:64 + mu + jb // 2, :] = \
                z[:, jb // 2:jb].reshape(_N, jb // 2, _N)
            off += jb * _N
    return _assemble(y_full, B, C)


# revision 15
# speedup vs baseline: 1.0794x; 1.0794x over previous
"""Trainium2 Bass kernel for a separable 3D Haar DWT (nn_DWT3D).

Problem: x [2, 128, 128, 128, 4] fp32, A [128, 128] (orthonormal Haar
analysis filter bank, 2-tap stride-2). Output: subband concat
[2, 64, 64, 64, 32].

Strategy (8 NeuronCores):
- Data-parallel over (batch, channel): B*C = 8 independent [128,128,128]
  volume transforms, one per core. Host deinterleaves channels on the
  way in and assembles the subband concat on the way out (numpy).
- Per core, separable transform = three 1D passes, ordered to decouple
  the engines (passes commute):
    * k-axis: DVE butterfly directly on the DMA'd input (no PE dep, so
      the vector engine starts as soon as the first chunk lands),
    * i-axis (partition axis): dense PE matmul, lhsT = h^2 * A^T (the
      j/k butterfly scales folded into the weights), ACT drains PSUM,
    * j-axis: DVE butterfly, software-pipelined one group behind so the
      vector engine never head-of-line blocks on the matmul.
- Streamed in ~1 MB groups; input DMA on the sync ring, output DMA on
  the scalar ring so the streams don't block each other.
"""

import numpy as np

_N = 128
_CORES = 8
# j-counts per streamed compute group (each even, sum to 128).
_GROUPS = [8] + [16] * 7 + [4, 4]

_cache = {}


def _build():
    import concourse.mybir as mybir
    from concourse import bacc
    from concourse.tile import TileContext

    nc = bacc.Bacc("TRN2", target_bir_lowering=False, debug=False,
                   num_devices=_CORES)
    v = nc.dram_tensor("v", [_N, _N * _N], mybir.dt.float32,
                       kind="ExternalInput")
    w = nc.dram_tensor("w", [_N, _N], mybir.dt.float32,
                       kind="ExternalInput")
    y = nc.dram_tensor("y", [_N, _N * _N], mybir.dt.float32,
                       kind="ExternalOutput")
    f32 = mybir.dt.float32

    groups = []
    off = 0
    for jb in _GROUPS:
        groups.append((off, jb))
        off += jb * _N
    assert off == _N * _N

    with TileContext(nc) as tc:
        with (
            tc.tile_pool(name="wpool", bufs=1) as wpool,
            tc.tile_pool(name="vin", bufs=4) as vpool,
            tc.tile_pool(name="y0", bufs=3) as y0pool,
            tc.tile_pool(name="y1", bufs=3) as y1pool,
            tc.tile_pool(name="y2", bufs=3) as y2pool,
            tc.tile_pool(name="ps", bufs=2, space="PSUM") as pspool,
        ):
            wt = wpool.tile([_N, _N], f32)
            nc.sync.dma_start(out=wt[:], in_=w[:])

            def emit_j_and_out(y1, off, jb):
                cw = jb * _N
                half = cw // 2
                # y1 free layout: (lhk, j, kk). j-butterfly over j.
                y1v = y1[:].rearrange("p (l j k) -> p l j k", l=2, k=64)
                y2 = y2pool.tile([_N, cw], f32, tag="y2")
                y2L = y2[:, 0:half].rearrange("p (l j k) -> p l j k",
                                              l=2, k=64)
                y2H = y2[:, half:cw].rearrange("p (l j k) -> p l j k",
                                               l=2, k=64)
                nc.vector.tensor_add(out=y2L[:],
                                     in0=y1v[:, :, 1:jb:2, :],
                                     in1=y1v[:, :, 0:jb:2, :])
                nc.vector.tensor_sub(out=y2H[:],
                                     in0=y1v[:, :, 1:jb:2, :],
                                     in1=y1v[:, :, 0:jb:2, :])
                nc.scalar.dma_start(out=y[:, off:off + cw], in_=y2[:])

            pending = None
            for off, jb in groups:
                cw = jb * _N
                half = cw // 2
                vin = vpool.tile([_N, cw], f32, tag="vin")
                nc.sync.dma_start(out=vin[:], in_=v[:, off:off + cw])

                # k-axis butterfly straight off the input: vin [i, (j, k)]
                # -> y0 [i, (lhk, j, kk)]
                vv = vin[:].rearrange("p (j k) -> p j k", k=_N)
                y0 = y0pool.tile([_N, cw], f32, tag="y0")
                y0L = y0[:, 0:half].rearrange("p (j k) -> p j k", k=64)
                y0H = y0[:, half:cw].rearrange("p (j k) -> p j k", k=64)
                nc.vector.tensor_add(out=y0L[:],
                                     in0=vv[:, :, 0:_N:2],
                                     in1=vv[:, :, 1:_N:2])
                nc.vector.tensor_sub(out=y0H[:],
                                     in0=vv[:, :, 1:_N:2],
                                     in1=vv[:, :, 0:_N:2])

                # i-axis transform on PE
                ps = pspool.tile([_N, cw], f32, tag="ps")
                for m in range(cw // 512):
                    nc.tensor.matmul(ps[:, m * 512:(m + 1) * 512], wt[:],
                                     y0[:, m * 512:(m + 1) * 512],
                                     start=True, stop=True)
                # ACT drains PSUM (frees the slot for the next group)
                y1 = y1pool.tile([_N, cw], f32, tag="y1")
                nc.scalar.copy(out=y1[:], in_=ps[:])

                if pending is not None:
                    emit_j_and_out(*pending)
                pending = (y1, off, jb)
            emit_j_and_out(*pending)

    nc.compile()
    return nc


def _get_nc():
    if "nc" not in _cache:
        _cache["nc"] = _build()
    return _cache["nc"]


def _haar_structure_ok(A):
    """A must be the 2-tap stride-2 filter bank with taps (h, h) lowpass /
    (-h, h) highpass, which is what the j/k butterflies hardcode."""
    if A.shape != (_N, _N):
        return False
    h = A[0, 0]
    if not np.isfinite(h) or abs(h) < 1e-8:
        return False
    expect = np.zeros((_N, _N), dtype=np.float32)
    for i in range(_N // 2):
        expect[i, 2 * i] = h
        expect[i, 2 * i + 1] = h
        expect[_N // 2 + i, 2 * i] = -h
        expect[_N // 2 + i, 2 * i + 1] = h
    return bool(np.allclose(A, expect, rtol=1e-5, atol=1e-7))


def _reference_host(x, A):
    """Generic numpy fallback (slow) for non-Haar A."""
    y = np.einsum("ai,nijkc->najkc", A, x, optimize=True)
    y = np.einsum("bj,najkc->nabkc", A, y, optimize=True)
    y = np.einsum("dk,nabkc->nabdc", A, y, optimize=True)
    return np.moveaxis(y, -1, 1)


def _assemble(y_full, B, C):
    """Slice transformed volumes y_full [B, C, 128,128,128] into the
    reference's subband concat [B, 64, 64, 64, 8*C] (incl. the duplicated
    HHH octant the reference produces)."""
    L, H = slice(0, 64), slice(64, 128)
    bands = [(L, L, L), (H, L, L), (L, H, L), (H, H, L),
             (L, L, H), (H, H, H), (L, H, H), (H, H, H)]
    out = np.empty((B, 64, 64, 64, 8 * C), dtype=np.float32)
    for s, (sa, sb, sd) in enumerate(bands):
        out[..., s * C:(s + 1) * C] = np.moveaxis(y_full[:, :, sa, sb, sd], 1, -1)
    return out


def kernel(x, A):
    from concourse.bass_utils import run_bass_kernel_spmd

    x = np.asarray(x, dtype=np.float32)
    A = np.asarray(A, dtype=np.float32)
    B, _, _, _, C = x.shape
    assert (B, C) == (2, 4) and x.shape[1:4] == (_N, _N, _N)

    if not _haar_structure_ok(A):
        return _assemble(_reference_host(x, A), B, C)

    h = float(A[0, 0])
    # PE weights: actual A with the j/k butterfly scale h^2 folded in.
    w = np.ascontiguousarray((h * h) * A.T)

    xs = np.ascontiguousarray(np.transpose(x, (0, 4, 1, 2, 3)))
    xs = xs.reshape(_CORES, _N, _N * _N)
    in_maps = [{"v": xs[g], "w": w} for g in range(_CORES)]

    nc = _get_nc()
    res = run_bass_kernel_spmd(nc, in_maps, list(range(_CORES)))

    # Per-core output, per group: [a, lhj, lhk, tj, kk]:
    # j' = jbase/2 + tj (lhj=0) or 64 + jbase/2 + tj; d = 64*lhk + kk.
    y_full = np.empty((B, C, _N, _N, _N), dtype=np.float32)
    for g in range(_CORES):
        cols = res.results[g]["y"]
        vol = y_full[g // C, g % C]
        off = 0
        for jb in _GROUPS:
            z = cols[:, off:off + jb * _N].reshape(_N, 2, 2, jb // 2, 64)
            z = z.transpose(0, 1, 3, 2, 4)  # [a, lhj, tj, lhk, kk]
            mu = (off // _N) // 2
            vol[:, mu:mu + jb // 2, :] = z[:, 0].reshape(_N, jb // 2, _N)
            vol[:, 64 + mu:64 + mu + jb // 2, :] = \
                z[:, 1].reshape(_N, jb // 2, _N)
            off += jb * _N
    return _assemble(y_full, B, C)
